# revision 5
# baseline (speedup 1.0000x reference)
"""Polar encoder (N=1024, K=512, batch=65536) on 8 Trainium2 NeuronCores.

Math: with frozen positions 0..511 (info_pos = 512..1023) and the reference's
stage-wise butterfly indices, the output is two identical copies of the
9-stage GF(2) superset-XOR butterfly applied to u's 512 columns:
    y[:, j] = XOR_{k superset of j} u[:, k]        (9-bit index lattice)
    out = concat([y, y], axis=1)
Values are 0.0/1.0 float32; bitwise XOR of their fp32 bit patterns
(0x00000000 / 0x3F800000) implements mod-2 addition with no casts.

Sharding: embarrassingly data-parallel over batch rows — 8192 rows/core.
Per core: load [128, 8*512] fp32 super-tiles, 9 in-place strided XOR stages
on VectorE (split with GpSimd), store the result twice (cols 0:512, 512:1024).
"""
import numpy as np
from contextlib import ExitStack

import concourse.bacc as bacc
import concourse.tile as tile
import concourse.mybir as mybir
from concourse.bass_utils import run_bass_kernel_spmd

N_CORES = 8
BATCH = 65536
K = 512            # butterfly width / info bits
N = 1024           # codeword length
ROWS = BATCH // N_CORES   # 8192 rows per core
P = 128            # SBUF partitions
G = 8              # row-groups per super-tile -> tile [128, G*K] fp32
NB_STAGES = 9
# row-groups per super-tile whose 9-stage chain runs on GpSimd instead of
# VectorE (groups are independent batch rows, so the split is truly parallel)
GP_GROUPS = 0
# run the butterfly in bf16 (2x DVE mode for stages 1-8); ScalarE does the casts
BF16 = False

_nc_cache = {}


def _build_program():
    nc = bacc.Bacc("TRN2", target_bir_lowering=False, debug=False)
    u_h = nc.dram_tensor("u", [ROWS, K], mybir.dt.float32, kind="ExternalInput")
    out_h = nc.dram_tensor("out", [ROWS, N], mybir.dt.float32, kind="ExternalOutput")
    with tile.TileContext(nc) as tc:
        _body(tc, out_h.ap(), u_h.ap())
    nc.compile()
    return nc


def _body(tc, out_ap, u_ap):
    nc = tc.nc
    W = G * K
    n_super = ROWS // (P * G)
    with ExitStack() as ctx:
        pool = ctx.enter_context(tc.tile_pool(name="data", bufs=3))
        for t in range(n_super):
            r0 = t * G * P
            tl = pool.tile([P, W], mybir.dt.float32)
            # one 2 MiB load: HBM rows r0..r0+G*128 -> [128, G*512]
            src = u_ap[r0:r0 + G * P, :].rearrange("(g p) k -> p g k", g=G)
            nc.sync.dma_start(out=tl[:, :].rearrange("p (g k) -> p g k", g=G), in_=src)
            if BF16:
                tb = pool.tile([P, W], mybir.dt.bfloat16, tag="bfwork")
                nc.scalar.copy(out=tb[:, :], in_=tl[:, :])
                x = tb[:, :].bitcast(mybir.dt.uint16)
            else:
                x = tl[:, :].bitcast(mybir.dt.uint32)
            ndve = G - GP_GROUPS
            for s in range(NB_STAGES):
                blk = 1 << s
                for eng, lo, hi in ((nc.vector, 0, ndve), (nc.gpsimd, ndve, G)):
                    if lo == hi:
                        continue
                    view = x[:, lo * K:hi * K].rearrange("p (b t l) -> p b t l",
                                                         t=2, l=blk)
                    dest = view[:, :, 0, :]
                    srcv = view[:, :, 1, :]
                    eng.tensor_tensor(out=dest, in0=dest, in1=srcv,
                                      op=mybir.AluOpType.bitwise_xor)
            if BF16:
                nc.scalar.copy(out=tl[:, :], in_=tb[:, :])
            # two 2 MiB stores: same tile into cols 0:K and K:N
            tl3 = tl[:, :].rearrange("p (g k) -> p g k", g=G)
            dst_lo = out_ap[r0:r0 + G * P, 0:K].rearrange("(g p) k -> p g k", g=G)
            dst_hi = out_ap[r0:r0 + G * P, K:N].rearrange("(g p) k -> p g k", g=G)
            nc.scalar.dma_start(out=dst_lo, in_=tl3)
            nc.scalar.dma_start(out=dst_hi, in_=tl3)


def _get_program():
    if "nc" not in _nc_cache:
        _nc_cache["nc"] = _build_program()
    return _nc_cache["nc"]


def _expected_ind_gather():
    ind = np.full((10, N + 1), N, dtype=np.int32)
    for s in range(10):
        rng = np.arange(N // 2)
        dest = rng * 2 - np.mod(rng, 2 ** s)
        ind[s, dest] = dest + 2 ** s
    return ind


def _inputs_match_expected(info_pos, ind_gather):
    try:
        return (info_pos.shape == (K,)
                and np.array_equal(info_pos, np.arange(K, N, dtype=np.int64).astype(info_pos.dtype))
                and ind_gather.shape == (10, N + 1)
                and np.array_equal(ind_gather, _expected_ind_gather()))
    except Exception:
        return False


def _numpy_fallback(u, info_pos, ind_gather):
    b = u.shape[0]
    c = np.zeros((b, N), u.dtype)
    c[:, np.asarray(info_pos)] = u
    x = np.concatenate([c, np.zeros((b, 1), u.dtype)], axis=1)
    for s in range(ind_gather.shape[0]):
        x = np.mod(x + x[:, np.asarray(ind_gather[s])], 2.0)
    return np.ascontiguousarray(x[:, :N]).astype(np.float32)


def _run(u, trace=False, **kw):
    nc = _get_program()
    u = np.ascontiguousarray(np.asarray(u, dtype=np.float32))
    in_maps = [{"u": u[c * ROWS:(c + 1) * ROWS]} for c in range(N_CORES)]
    res = run_bass_kernel_spmd(nc, in_maps, list(range(N_CORES)), trace=trace, **kw)
    out = np.concatenate([res.results[c]["out"] for c in range(N_CORES)], axis=0)
    return out, res


def kernel(u, info_pos, ind_gather):
    if not _inputs_match_expected(np.asarray(info_pos), np.asarray(ind_gather)):
        return _numpy_fallback(np.asarray(u), info_pos, ind_gather)
    out, _ = _run(u)
    return out


# revision 15
# speedup vs baseline: 1241045.6145x; 1241045.6145x over previous
"""Polar encoder (N=1024, K=512, batch=65536) on 8 Trainium2 NeuronCores.

Math: with frozen positions 0..511 (info_pos = 512..1023) and the reference's
stage-wise butterfly indices, the output is two identical copies of the
9-stage GF(2) superset-XOR butterfly applied to u's 512 columns:
    y[:, j] = XOR_{k superset of j} u[:, k]        (9-bit index lattice)
    out = concat([y, y], axis=1)
Values are 0.0/1.0 float32; bitwise XOR of their fp32 bit patterns
(0x00000000 / 0x3F800000) implements mod-2 addition with no casts.

Sharding: embarrassingly data-parallel over batch rows — 8192 rows/core.
Per core: load [128, 8*512] fp32 super-tiles (one 2 MiB DMA each), cast to
bf16 on ScalarE, run the 9 in-place strided XOR stages on VectorE as
`not_equal` tensor_tensor ops (== XOR for 0/1; bf16 unit-stride innermost
dims hit the 2x_1P DVE perf mode for stages 1-8), cast back to fp32 on
ScalarE, store the tile twice (cols 0:512 and 512:1024). ~151 us/core on HW
(loop-slope measured) vs a ~140 us HBM roofline (16 MiB read + 32 MiB
written per core at ~358 GB/s).
"""
import numpy as np
from contextlib import ExitStack

import concourse.bacc as bacc
import concourse.tile as tile
import concourse.mybir as mybir
from concourse.bass_utils import run_bass_kernel_spmd

N_CORES = 8
BATCH = 65536
K = 512            # butterfly width / info bits
N = 1024           # codeword length
ROWS = BATCH // N_CORES   # 8192 rows per core
P = 128            # SBUF partitions
G = 8              # row-groups per super-tile -> tile [128, G*K] fp32
NB_STAGES = 9
# row-groups per super-tile whose 9-stage chain runs on GpSimd instead of
# VectorE. Keep 0: neuronxcc rejects TensorTensor on the Pool engine (ISA v3).
GP_GROUPS = 0
# run the butterfly in bf16 (2x_1P DVE mode for stages 1-8); ScalarE casts
BF16 = True
# tile-pool buffers (pipeline depth); 6*(16+8) KiB/partition fits SBUF
BUFS = 6
# not_equal (float compare; == XOR for 0/1 values) instead of bitwise xor --
# neuronxcc only supports bitwise ops on DVE for 32-bit ints, not 16-bit
DVE_NE = True

_nc_cache = {}


def _build_program(loops=0):
    """loops=0: plain program (graded path). loops=L>0: wrap the body in a
    hardware For_i repeating it L times — used only for timing measurements."""
    nc = bacc.Bacc("TRN2", target_bir_lowering=False, debug=False)
    u_h = nc.dram_tensor("u", [ROWS, K], mybir.dt.float32, kind="ExternalInput")
    out_h = nc.dram_tensor("out", [ROWS, N], mybir.dt.float32, kind="ExternalOutput")
    with tile.TileContext(nc) as tc:
        if loops:
            with tc.For_i(0, loops, 1):
                _body(tc, out_h.ap(), u_h.ap())
        else:
            _body(tc, out_h.ap(), u_h.ap())
    nc.compile()
    return nc


def _body(tc, out_ap, u_ap):
    nc = tc.nc
    W = G * K
    n_super = ROWS // (P * G)
    DVE_OP = mybir.AluOpType.not_equal if DVE_NE else mybir.AluOpType.bitwise_xor
    with ExitStack() as ctx:
        pool = ctx.enter_context(tc.tile_pool(name="data", bufs=BUFS))
        for t in range(n_super):
            r0 = t * G * P
            tl = pool.tile([P, W], mybir.dt.float32)
            # one 2 MiB load: HBM rows r0..r0+G*128 -> [128, G*512]
            src = u_ap[r0:r0 + G * P, :].rearrange("(g p) k -> p g k", g=G)
            nc.sync.dma_start(out=tl[:, :].rearrange("p (g k) -> p g k", g=G), in_=src)
            if BF16:
                work = pool.tile([P, W], mybir.dt.bfloat16, tag="bfwork")
                nc.scalar.copy(out=work[:, :], in_=tl[:, :])
                x_dve = (work[:, :] if DVE_NE
                         else work[:, :].bitcast(mybir.dt.uint16))
            else:
                work = tl
                x_dve = (work[:, :] if DVE_NE
                         else work[:, :].bitcast(mybir.dt.uint32))
            # GpSimd can't do integer bitwise ops; for 0/1 values XOR == not_equal
            x_gp = work[:, :]
            ndve = G - GP_GROUPS
            for s in range(NB_STAGES):
                blk = 1 << s
                for x, op, lo, hi, eng in (
                    (x_dve, DVE_OP, 0, ndve, nc.vector),
                    (x_gp, mybir.AluOpType.not_equal, ndve, G, nc.gpsimd),
                ):
                    if lo == hi:
                        continue
                    view = x[:, lo * K:hi * K].rearrange("p (b t l) -> p b t l",
                                                         t=2, l=blk)
                    dest = view[:, :, 0, :]
                    srcv = view[:, :, 1, :]
                    eng.tensor_tensor(out=dest, in0=dest, in1=srcv, op=op)
            if BF16:
                nc.scalar.copy(out=tl[:, :], in_=work[:, :])
            # two 2 MiB stores: same tile into cols 0:K and K:N
            tl3 = tl[:, :].rearrange("p (g k) -> p g k", g=G)
            dst_lo = out_ap[r0:r0 + G * P, 0:K].rearrange("(g p) k -> p g k", g=G)
            dst_hi = out_ap[r0:r0 + G * P, K:N].rearrange("(g p) k -> p g k", g=G)
            nc.scalar.dma_start(out=dst_lo, in_=tl3)
            nc.scalar.dma_start(out=dst_hi, in_=tl3)


def _get_program():
    if "nc" not in _nc_cache:
        _nc_cache["nc"] = _build_program()
    return _nc_cache["nc"]


def _expected_ind_gather():
    ind = np.full((10, N + 1), N, dtype=np.int32)
    for s in range(10):
        rng = np.arange(N // 2)
        dest = rng * 2 - np.mod(rng, 2 ** s)
        ind[s, dest] = dest + 2 ** s
    return ind


def _inputs_match_expected(info_pos, ind_gather):
    try:
        return (info_pos.shape == (K,)
                and np.array_equal(info_pos, np.arange(K, N, dtype=np.int64).astype(info_pos.dtype))
                and ind_gather.shape == (10, N + 1)
                and np.array_equal(ind_gather, _expected_ind_gather()))
    except Exception:
        return False


def _numpy_fallback(u, info_pos, ind_gather):
    b = u.shape[0]
    c = np.zeros((b, N), u.dtype)
    c[:, np.asarray(info_pos)] = u
    x = np.concatenate([c, np.zeros((b, 1), u.dtype)], axis=1)
    for s in range(ind_gather.shape[0]):
        x = np.mod(x + x[:, np.asarray(ind_gather[s])], 2.0)
    return np.ascontiguousarray(x[:, :N]).astype(np.float32)


def _run(u, trace=False, **kw):
    nc = _get_program()
    u = np.ascontiguousarray(np.asarray(u, dtype=np.float32))
    in_maps = [{"u": u[c * ROWS:(c + 1) * ROWS]} for c in range(N_CORES)]
    res = run_bass_kernel_spmd(nc, in_maps, list(range(N_CORES)), trace=trace, **kw)
    out = np.concatenate([res.results[c]["out"] for c in range(N_CORES)], axis=0)
    return out, res


def kernel(u, info_pos, ind_gather):
    u = np.asarray(u)
    if (u.shape != (BATCH, K)
            or not _inputs_match_expected(np.asarray(info_pos),
                                          np.asarray(ind_gather))):
        return _numpy_fallback(u, info_pos, ind_gather)
    out, _ = _run(u)
    return out
